# revision 12
# baseline (speedup 1.0000x reference)
"""Trainium2 Bass kernel for nn_ApplyAttentionPolicyMap.

Reference computes out = concat(logits, pp_logits) @ fc1 where fc1 is a
4288x1858 one-hot column-selection map: out[b, j] = flat[b, sel[j]].

Strategy (8 NeuronCores, data-parallel over batch):
  * Host: shard the batch 8-ways; each core's activation shard is laid out
    feature-major (xT [4288, 1024]) so the selection becomes a row gather.
    fc1 is reduced to its sparse index form sel[1858] (as the sharding hint
    suggests) and replicated to every core as an int32 index tensor.
  * Device, pipelined per 128-column chunk (15 chunks):
      - gpsimd indirect_dma_start gathers the chunk's 128 selected feature
        rows from HBM into SBUF ([j%128 partition, 1024 batch]);
      - the PE transposes each [128,128] block back to batch-major via
        identity matmul into rotating PSUM banks;
      - DVE/ACT evacuate PSUM into the output staging tile;
      - HWDGE (Sync) writes the chunk's columns of the row-major
        [1024, 1858] shard to DRAM.
"""

import numpy as np

import concourse.bacc as bacc
import concourse.bass as bass
import concourse.mybir as mybir
from concourse.bass_utils import run_bass_kernel_spmd

N_CORES = 8
B = 8192
B_SHARD = B // N_CORES            # 1024
IN_DIM = 64 * 64 + 8 * 24         # 4288
OUT_DIM = 1858
N_BTILE = B_SHARD // 128          # 8 batch sub-tiles per core
N_CHUNK = 15                      # ceil(1858/128) output column chunks
NUM_IDX = N_CHUNK * 128           # 1920 padded gather indices

_DT = mybir.dt.float32

_cached = {}


def _build_nc():
    nc = bacc.Bacc("TRN2")
    xT = nc.declare_dram_parameter("xT", [IN_DIM, B_SHARD], _DT, isOutput=False)
    idx_d = nc.declare_dram_parameter("idx", [128, N_CHUNK], mybir.dt.int32, isOutput=False)
    ident_d = nc.declare_dram_parameter("ident", [128, 128], _DT, isOutput=False)
    out_d = nc.declare_dram_parameter("out", [B_SHARD, OUT_DIM], _DT, isOutput=True)

    # DRAM view of out with batch sub-tile explicit: partition = row within
    # sub-tile, free dims = (sub-tile, column).
    out_v = out_d[:, :].rearrange("(t p) n -> p t n", p=128)

    from contextlib import ExitStack

    with (
        nc.sbuf_tensor("gath", [128, N_CHUNK, B_SHARD], _DT) as gath,
        nc.sbuf_tensor("outb", [128, N_BTILE, NUM_IDX], _DT) as outb,
        nc.sbuf_tensor("idx_sb", [128, N_CHUNK], mybir.dt.int32) as idx_sb,
        nc.sbuf_tensor("ident_sb", [128, 128], _DT) as ident_sb,
        nc.psum_tensor("pt", [128, 8, 512], _DT) as pt,
        nc.semaphore("io") as io_sem,
        nc.semaphore("ident_io") as ident_sem,
        nc.semaphore("mm") as mm_sem,
        nc.semaphore("dve") as dve_sem,
        nc.semaphore("act") as act_sem,
        nc.semaphore("outs") as out_sem,
        ExitStack() as stack,
        nc.Block() as block,
    ):
        gsem = [stack.enter_context(nc.semaphore(f"g{c}")) for c in range(N_CHUNK)]  # noqa: ANT232

        @block.gpsimd
        def _(g):
            # idx load via SWDGE so it's ready as soon as possible
            g.dma_start(idx_sb[:, :], idx_d[:, :]).then_inc(io_sem, 16)
            g.wait_ge(io_sem, 16)
            for c in range(N_CHUNK):
                g.indirect_dma_start(
                    out=gath[:, c, :],
                    out_offset=None,
                    in_=xT[:, :],
                    in_offset=bass.IndirectOffsetOnAxis(
                        ap=idx_sb[:, c : c + 1], axis=0
                    ),
                ).then_inc(gsem[c], 16)

        @block.tensor
        def _(t):
            t.wait_ge(ident_sem, 16)  # identity loaded
            for c in range(N_CHUNK):
                t.wait_ge(gsem[c], 16)
                for bb in range(N_BTILE):
                    k = c * N_BTILE + bb
                    bank = k % 8
                    if k >= 8:
                        m = k - 8
                        if bank < 4:
                            t.wait_ge(dve_sem, (m // 8) * 4 + bank + 1)
                        else:
                            t.wait_ge(act_sem, (m // 8) * 4 + (bank - 4) + 1)
                    t.matmul(
                        pt[:, bank, 0:128],
                        gath[:, c, bb * 128 : (bb + 1) * 128],
                        ident_sb[:, :],
                        is_transpose=True,
                        start=True,
                        stop=True,
                    ).then_inc(mm_sem, 1)

        @block.vector
        def _(v):
            for m in range(N_CHUNK * N_BTILE):
                if m % 8 >= 4:
                    continue
                c, bb = m // 8, m % 8
                v.wait_ge(mm_sem, m + 1)
                v.tensor_copy(
                    out=outb[:, bb, c * 128 : (c + 1) * 128],
                    in_=pt[:, m % 8, 0:128],
                ).then_inc(dve_sem, 1)

        @block.scalar
        def _(s):
            for m in range(N_CHUNK * N_BTILE):
                if m % 8 < 4:
                    continue
                c, bb = m // 8, m % 8
                s.wait_ge(mm_sem, m + 1)
                s.copy(
                    out=outb[:, bb, c * 128 : (c + 1) * 128],
                    in_=pt[:, m % 8, 0:128],
                ).then_inc(act_sem, 1)

        @block.sync
        def _(s):
            s.dma_start(ident_sb[:, :], ident_d[:, :]).then_inc(ident_sem, 16)
            for c in range(N_CHUNK):
                s.wait_ge(dve_sem, 4 * (c + 1))
                s.wait_ge(act_sem, 4 * (c + 1))
                col0 = c * 128
                col1 = min(col0 + 128, OUT_DIM)
                s.dma_start(
                    out=out_v[:, :, col0:col1],
                    in_=outb[:, :, col0:col1],
                ).then_inc(out_sem, 16)
            s.wait_ge(out_sem, 16 * N_CHUNK)

    nc.compile()
    return nc


def _get_nc():
    if "nc" not in _cached:
        _cached["nc"] = _build_nc()
    return _cached["nc"]


def _extract_sel(fc1: np.ndarray):
    """Return sel[j] with fc1 == one_hot(sel), or None if fc1 is not an
    exact one-hot column-selection map."""
    if fc1.shape != (IN_DIM, OUT_DIM):
        return None
    sel = np.argmax(fc1, axis=0)
    ok = (fc1[sel, np.arange(OUT_DIM)] == 1.0).all()
    if not ok:
        return None
    # each column must have exactly one nonzero
    nnz = np.count_nonzero(fc1, axis=0)
    if not (nnz == 1).all():
        return None
    return sel.astype(np.int64)


def _build_idx_tensor(sel: np.ndarray) -> np.ndarray:
    """int32 [128, N_CHUNK]: idx[p, c] = sel[c*128 + p] (0 for padding)."""
    sel_pad = np.zeros(NUM_IDX, dtype=np.int32)
    sel_pad[:OUT_DIM] = sel.astype(np.int32)
    return sel_pad.reshape(N_CHUNK, 128).T.copy()


def kernel(logits: np.ndarray, pp_logits: np.ndarray, fc1: np.ndarray) -> np.ndarray:
    logits = np.asarray(logits, dtype=np.float32)
    pp_logits = np.asarray(pp_logits, dtype=np.float32)
    fc1 = np.asarray(fc1, dtype=np.float32)
    b = logits.shape[0]
    flat = np.concatenate(
        [logits.reshape(b, 64 * 64), pp_logits.reshape(b, 8 * 24)], axis=1
    )

    sel = _extract_sel(fc1)
    if sel is None or b != B:
        # Degenerate input (fc1 not an exact selection map, or unexpected
        # batch) — fall back to the dense reference computation.
        return flat @ fc1

    nc = _get_nc()
    idx_np = _build_idx_tensor(sel)
    ident_np = np.eye(128, dtype=np.float32)
    xT = np.ascontiguousarray(flat.T)  # [4288, 8192]

    in_maps = []
    for i in range(N_CORES):
        shard = np.ascontiguousarray(xT[:, i * B_SHARD : (i + 1) * B_SHARD])
        in_maps.append({"xT": shard, "idx": idx_np, "ident": ident_np})

    res = run_bass_kernel_spmd(nc, in_maps, list(range(N_CORES)))
    out = np.concatenate([res.results[i]["out"] for i in range(N_CORES)], axis=0)
    return np.ascontiguousarray(out.astype(np.float32))


# revision 13
# speedup vs baseline: 1.3275x; 1.3275x over previous
"""Trainium2 Bass kernel for nn_ApplyAttentionPolicyMap.

Reference computes out = concat(logits, pp_logits) @ fc1 where fc1 is a
4288x1858 one-hot column-selection map: out[b, j] = flat[b, sel[j]].

Strategy (8 NeuronCores, data-parallel over batch):
  * Host: shard the batch 8-ways; each core's activation shard is laid out
    feature-major (xT [4288, 1024]) so the selection becomes a row gather.
    fc1 is reduced to its sparse index form sel[1858] (as the sharding hint
    suggests) and replicated to every core as an int32 index tensor.
  * Device, pipelined per 128-column chunk (15 chunks):
      - gpsimd indirect_dma_start gathers the chunk's 128 selected feature
        rows from HBM into SBUF ([j%128 partition, 1024 batch]);
      - the PE transposes each [128,128] block back to batch-major via
        identity matmul into rotating PSUM banks;
      - DVE/ACT evacuate PSUM into the output staging tile;
      - HWDGE (Sync) writes the chunk's columns of the row-major
        [1024, 1858] shard to DRAM.
"""

import numpy as np

import concourse.bacc as bacc
import concourse.bass as bass
import concourse.mybir as mybir
from concourse.bass_utils import run_bass_kernel_spmd

N_CORES = 8
B = 8192
B_SHARD = B // N_CORES            # 1024
IN_DIM = 64 * 64 + 8 * 24         # 4288
OUT_DIM = 1858
N_BTILE = B_SHARD // 128          # 8 batch sub-tiles per core
N_CHUNK = 15                      # ceil(1858/128) output column chunks
NUM_IDX = N_CHUNK * 128           # 1920 padded gather indices

_DT = mybir.dt.float32

_cached = {}


def _build_nc():
    nc = bacc.Bacc("TRN2")
    xT = nc.declare_dram_parameter("xT", [IN_DIM, B_SHARD], _DT, isOutput=False)
    idx_d = nc.declare_dram_parameter("idx", [128, N_CHUNK], mybir.dt.int32, isOutput=False)
    ident_d = nc.declare_dram_parameter("ident", [128, 128], _DT, isOutput=False)
    out_d = nc.declare_dram_parameter("out", [B_SHARD, OUT_DIM], _DT, isOutput=True)

    # DRAM view of out with batch sub-tile explicit: partition = row within
    # sub-tile, free dims = (sub-tile, column).
    out_v = out_d[:, :].rearrange("(t p) n -> p t n", p=128)

    from contextlib import ExitStack

    with (
        nc.sbuf_tensor("gath", [128, N_CHUNK, B_SHARD], _DT) as gath,
        nc.sbuf_tensor("outb", [128, N_BTILE, NUM_IDX], _DT) as outb,
        nc.sbuf_tensor("idx_sb", [128, N_CHUNK], mybir.dt.int32) as idx_sb,
        nc.sbuf_tensor("ident_sb", [128, 128], _DT) as ident_sb,
        nc.psum_tensor("pt", [128, 8, 512], _DT) as pt,
        nc.semaphore("io") as io_sem,
        nc.semaphore("ident_io") as ident_sem,
        nc.semaphore("mm") as mm_sem,
        nc.semaphore("dve") as dve_sem,
        nc.semaphore("act") as act_sem,
        nc.semaphore("outs") as out_sem,
        ExitStack() as stack,
        nc.Block() as block,
    ):
        gsem = [stack.enter_context(nc.semaphore(f"g{c}")) for c in range(N_CHUNK)]  # noqa: ANT232

        @block.gpsimd
        def _(g):
            # idx load via SWDGE so it's ready as soon as possible
            g.dma_start(idx_sb[:, :], idx_d[:, :]).then_inc(io_sem, 16)
            g.wait_ge(io_sem, 16)
            for c in range(N_CHUNK):
                g.indirect_dma_start(
                    out=gath[:, c, :],
                    out_offset=None,
                    in_=xT[:, :],
                    in_offset=bass.IndirectOffsetOnAxis(
                        ap=idx_sb[:, c : c + 1], axis=0
                    ),
                ).then_inc(gsem[c], 16)

        @block.tensor
        def _(t):
            t.wait_ge(ident_sem, 16)  # identity loaded
            for c in range(N_CHUNK):
                t.wait_ge(gsem[c], 16)
                for bb in range(N_BTILE):
                    k = c * N_BTILE + bb
                    bank = k % 8
                    if k >= 8:
                        m = k - 8
                        if bank < 4:
                            t.wait_ge(dve_sem, (m // 8) * 4 + bank + 1)
                        else:
                            t.wait_ge(act_sem, (m // 8) * 4 + (bank - 4) + 1)
                    t.matmul(
                        pt[:, bank, 0:128],
                        gath[:, c, bb * 128 : (bb + 1) * 128],
                        ident_sb[:, :],
                        is_transpose=True,
                        start=True,
                        stop=True,
                    ).then_inc(mm_sem, 1)

        @block.vector
        def _(v):
            for m in range(N_CHUNK * N_BTILE):
                if m % 8 >= 4:
                    continue
                c, bb = m // 8, m % 8
                v.wait_ge(mm_sem, m + 1)
                v.tensor_copy(
                    out=outb[:, bb, c * 128 : (c + 1) * 128],
                    in_=pt[:, m % 8, 0:128],
                ).then_inc(dve_sem, 1)

        @block.scalar
        def _(s):
            for m in range(N_CHUNK * N_BTILE):
                if m % 8 < 4:
                    continue
                c, bb = m // 8, m % 8
                s.wait_ge(mm_sem, m + 1)
                s.copy(
                    out=outb[:, bb, c * 128 : (c + 1) * 128],
                    in_=pt[:, m % 8, 0:128],
                ).then_inc(act_sem, 1)

        @block.sync
        def _(s):
            # Output DMA in groups of several chunks: per-partition DRAM runs
            # of >=1.5KB keep the HWDGE descriptors at line rate (512B
            # descriptors from single-chunk stores run at ~60% efficiency).
            out_groups = [4, 4, 4, 3]
            s.dma_start(ident_sb[:, :], ident_d[:, :]).then_inc(ident_sem, 16)
            c_end = 0
            for sz in out_groups:
                c0, c_end = c_end, c_end + sz
                s.wait_ge(dve_sem, 4 * c_end)
                s.wait_ge(act_sem, 4 * c_end)
                col0 = c0 * 128
                col1 = min(c_end * 128, OUT_DIM)
                s.dma_start(
                    out=out_v[:, :, col0:col1],
                    in_=outb[:, :, col0:col1],
                ).then_inc(out_sem, 16)
            s.wait_ge(out_sem, 16 * len(out_groups))

    nc.compile()
    return nc


def _get_nc():
    if "nc" not in _cached:
        _cached["nc"] = _build_nc()
    return _cached["nc"]


def _extract_sel(fc1: np.ndarray):
    """Return sel[j] with fc1 == one_hot(sel), or None if fc1 is not an
    exact one-hot column-selection map."""
    if fc1.shape != (IN_DIM, OUT_DIM):
        return None
    sel = np.argmax(fc1, axis=0)
    ok = (fc1[sel, np.arange(OUT_DIM)] == 1.0).all()
    if not ok:
        return None
    # each column must have exactly one nonzero
    nnz = np.count_nonzero(fc1, axis=0)
    if not (nnz == 1).all():
        return None
    return sel.astype(np.int64)


def _build_idx_tensor(sel: np.ndarray) -> np.ndarray:
    """int32 [128, N_CHUNK]: idx[p, c] = sel[c*128 + p] (0 for padding)."""
    sel_pad = np.zeros(NUM_IDX, dtype=np.int32)
    sel_pad[:OUT_DIM] = sel.astype(np.int32)
    return sel_pad.reshape(N_CHUNK, 128).T.copy()


def kernel(logits: np.ndarray, pp_logits: np.ndarray, fc1: np.ndarray) -> np.ndarray:
    logits = np.asarray(logits, dtype=np.float32)
    pp_logits = np.asarray(pp_logits, dtype=np.float32)
    fc1 = np.asarray(fc1, dtype=np.float32)
    b = logits.shape[0]
    flat = np.concatenate(
        [logits.reshape(b, 64 * 64), pp_logits.reshape(b, 8 * 24)], axis=1
    )

    sel = _extract_sel(fc1)
    if sel is None or b != B:
        # Degenerate input (fc1 not an exact selection map, or unexpected
        # batch) — fall back to the dense reference computation.
        return flat @ fc1

    nc = _get_nc()
    idx_np = _build_idx_tensor(sel)
    ident_np = np.eye(128, dtype=np.float32)
    xT = np.ascontiguousarray(flat.T)  # [4288, 8192]

    in_maps = []
    for i in range(N_CORES):
        shard = np.ascontiguousarray(xT[:, i * B_SHARD : (i + 1) * B_SHARD])
        in_maps.append({"xT": shard, "idx": idx_np, "ident": ident_np})

    res = run_bass_kernel_spmd(nc, in_maps, list(range(N_CORES)))
    out = np.concatenate([res.results[i]["out"] for i in range(N_CORES)], axis=0)
    return np.ascontiguousarray(out.astype(np.float32))


# revision 15
# speedup vs baseline: 1.3457x; 1.0137x over previous
"""Trainium2 Bass kernel for nn_ApplyAttentionPolicyMap.

Reference computes out = concat(logits, pp_logits) @ fc1 where fc1 is a
4288x1858 one-hot column-selection map: out[b, j] = flat[b, sel[j]].

Strategy (8 NeuronCores, data-parallel over batch):
  * Host: shard the batch 8-ways; each core's activation shard is laid out
    feature-major (xT [4288, 1024]) so the selection becomes a row gather.
    fc1 is reduced to its sparse index form sel[1858] (as the sharding hint
    suggests) and replicated to every core as an int32 index tensor.
  * Device, pipelined per 128-column chunk (15 chunks):
      - gpsimd indirect_dma_start gathers the chunk's 128 selected feature
        rows from HBM into SBUF ([j%128 partition, 1024 batch]);
      - the PE transposes each [128,128] block back to batch-major via
        identity matmul into rotating PSUM banks;
      - DVE/ACT evacuate PSUM into the output staging tile;
      - HWDGE (Sync) writes the chunk's columns of the row-major
        [1024, 1858] shard to DRAM.
"""

import numpy as np

import concourse.bacc as bacc
import concourse.bass as bass
import concourse.mybir as mybir
from concourse.bass_utils import run_bass_kernel_spmd

N_CORES = 8
B = 8192
B_SHARD = B // N_CORES            # 1024
IN_DIM = 64 * 64 + 8 * 24         # 4288
OUT_DIM = 1858
N_BTILE = B_SHARD // 128          # 8 batch sub-tiles per core
N_CHUNK = 15                      # ceil(1858/128) output column chunks
NUM_IDX = N_CHUNK * 128           # 1920 padded gather indices

_DT = mybir.dt.float32

_cached = {}


def _build_nc():
    nc = bacc.Bacc("TRN2")
    xT = nc.declare_dram_parameter("xT", [IN_DIM, B_SHARD], _DT, isOutput=False)
    idx_d = nc.declare_dram_parameter("idx", [128, N_CHUNK], mybir.dt.int32, isOutput=False)
    ident_d = nc.declare_dram_parameter("ident", [128, 128], _DT, isOutput=False)
    out_d = nc.declare_dram_parameter("out", [B_SHARD, OUT_DIM], _DT, isOutput=True)

    # DRAM view of out with batch sub-tile explicit: partition = row within
    # sub-tile, free dims = (sub-tile, column).
    out_v = out_d[:, :].rearrange("(t p) n -> p t n", p=128)

    from contextlib import ExitStack

    with (
        nc.sbuf_tensor("gath", [128, N_CHUNK, B_SHARD], _DT) as gath,
        nc.sbuf_tensor("outb", [128, N_BTILE, NUM_IDX], _DT) as outb,
        nc.sbuf_tensor("idx_sb", [128, N_CHUNK], mybir.dt.int32) as idx_sb,
        nc.sbuf_tensor("ident_sb", [128, 128], _DT) as ident_sb,
        nc.psum_tensor("pt", [128, 8, 512], _DT) as pt,
        nc.semaphore("io") as io_sem,
        nc.semaphore("ident_io") as ident_sem,
        nc.semaphore("mm") as mm_sem,
        nc.semaphore("dve") as dve_sem,
        nc.semaphore("act") as act_sem,
        nc.semaphore("outs") as out_sem,
        ExitStack() as stack,
        nc.Block() as block,
    ):
        gsem = [stack.enter_context(nc.semaphore(f"g{c}")) for c in range(N_CHUNK)]  # noqa: ANT232

        last_valid = OUT_DIM - (N_CHUNK - 1) * 128  # 66 rows in final chunk

        @block.gpsimd
        def _(g):
            g.dma_start(ident_sb[:, :], ident_d[:, :]).then_inc(ident_sem, 16)
            g.wait_ge(io_sem, 16)
            for c in range(N_CHUNK):
                np_ = 128 if c < N_CHUNK - 1 else last_valid
                g.indirect_dma_start(
                    out=gath[0:np_, c, :],
                    out_offset=None,
                    in_=xT[:, :],
                    in_offset=bass.IndirectOffsetOnAxis(
                        ap=idx_sb[0:np_, c : c + 1], axis=0
                    ),
                ).then_inc(gsem[c], 16)

        @block.tensor
        def _(t):
            t.wait_ge(ident_sem, 16)  # identity loaded
            for c in range(N_CHUNK):
                t.wait_ge(gsem[c], 16)
                for bb in range(N_BTILE):
                    k = c * N_BTILE + bb
                    bank = k % 8
                    if k >= 8:
                        m = k - 8
                        if bank < 4:
                            t.wait_ge(dve_sem, (m // 8) * 4 + bank + 1)
                        else:
                            t.wait_ge(act_sem, (m // 8) * 4 + (bank - 4) + 1)
                    t.matmul(
                        pt[:, bank, 0:128],
                        gath[:, c, bb * 128 : (bb + 1) * 128],
                        ident_sb[:, :],
                        is_transpose=True,
                        start=True,
                        stop=True,
                    ).then_inc(mm_sem, 1)

        @block.vector
        def _(v):
            for m in range(N_CHUNK * N_BTILE):
                if m % 8 >= 4:
                    continue
                c, bb = m // 8, m % 8
                v.wait_ge(mm_sem, m + 1)
                v.tensor_copy(
                    out=outb[:, bb, c * 128 : (c + 1) * 128],
                    in_=pt[:, m % 8, 0:128],
                ).then_inc(dve_sem, 1)

        @block.scalar
        def _(s):
            for m in range(N_CHUNK * N_BTILE):
                if m % 8 < 4:
                    continue
                c, bb = m // 8, m % 8
                s.wait_ge(mm_sem, m + 1)
                s.copy(
                    out=outb[:, bb, c * 128 : (c + 1) * 128],
                    in_=pt[:, m % 8, 0:128],
                ).then_inc(act_sem, 1)

        @block.sync
        def _(s):
            # Output DMA in groups of several chunks: per-partition DRAM runs
            # of >=1.5KB keep the HWDGE descriptors at line rate (512B
            # descriptors from single-chunk stores run at ~60% efficiency).
            out_groups = [4, 4, 4, 3]
            s.dma_start(idx_sb[:, :], idx_d[:, :]).then_inc(io_sem, 16)
            c_end = 0
            for sz in out_groups:
                c0, c_end = c_end, c_end + sz
                s.wait_ge(dve_sem, 4 * c_end)
                s.wait_ge(act_sem, 4 * c_end)
                col0 = c0 * 128
                col1 = min(c_end * 128, OUT_DIM)
                s.dma_start(
                    out=out_v[:, :, col0:col1],
                    in_=outb[:, :, col0:col1],
                ).then_inc(out_sem, 16)
            s.wait_ge(out_sem, 16 * len(out_groups))

    nc.compile()
    return nc


def _get_nc():
    if "nc" not in _cached:
        _cached["nc"] = _build_nc()
    return _cached["nc"]


def _extract_sel(fc1: np.ndarray):
    """Return sel[j] with fc1 == one_hot(sel), or None if fc1 is not an
    exact one-hot column-selection map."""
    if fc1.shape != (IN_DIM, OUT_DIM):
        return None
    sel = np.argmax(fc1, axis=0)
    ok = (fc1[sel, np.arange(OUT_DIM)] == 1.0).all()
    if not ok:
        return None
    # each column must have exactly one nonzero
    nnz = np.count_nonzero(fc1, axis=0)
    if not (nnz == 1).all():
        return None
    return sel.astype(np.int64)


def _build_idx_tensor(sel: np.ndarray) -> np.ndarray:
    """int32 [128, N_CHUNK]: idx[p, c] = sel[c*128 + p] (0 for padding)."""
    sel_pad = np.zeros(NUM_IDX, dtype=np.int32)
    sel_pad[:OUT_DIM] = sel.astype(np.int32)
    return sel_pad.reshape(N_CHUNK, 128).T.copy()


def kernel(logits: np.ndarray, pp_logits: np.ndarray, fc1: np.ndarray) -> np.ndarray:
    logits = np.asarray(logits, dtype=np.float32)
    pp_logits = np.asarray(pp_logits, dtype=np.float32)
    fc1 = np.asarray(fc1, dtype=np.float32)
    b = logits.shape[0]
    flat = np.concatenate(
        [logits.reshape(b, 64 * 64), pp_logits.reshape(b, 8 * 24)], axis=1
    )

    sel = _extract_sel(fc1)
    if sel is None or b != B:
        # Degenerate input (fc1 not an exact selection map, or unexpected
        # batch) — fall back to the dense reference computation.
        return flat @ fc1

    nc = _get_nc()
    idx_np = _build_idx_tensor(sel)
    ident_np = np.eye(128, dtype=np.float32)
    xT = np.ascontiguousarray(flat.T)  # [4288, 8192]

    in_maps = []
    for i in range(N_CORES):
        shard = np.ascontiguousarray(xT[:, i * B_SHARD : (i + 1) * B_SHARD])
        in_maps.append({"xT": shard, "idx": idx_np, "ident": ident_np})

    res = run_bass_kernel_spmd(nc, in_maps, list(range(N_CORES)))
    out = np.concatenate([res.results[i]["out"] for i in range(N_CORES)], axis=0)
    return np.ascontiguousarray(out.astype(np.float32))
